# revision 18
# baseline (speedup 1.0000x reference)
"""Trainium2 Bass kernel for ConvOffset: Conv2D(3x3, fixed one-hot-tap kernel) + Dense.

The staged conv kernel is zero everywhere except the center tap [1,1], which is
all-ones over (cin, cout).  Folding the conv kernel into the Dense weight W:

    out[b,h,w,o] = sum_i x[b,h,w,i] * M11[i,o] + bias @ W,
    M11[i,o]     = sum_c K[1,1,i,c] * W[c,o]

and because K[1,1] has identical rows (all-ones), M11 is rank-1 with identical
rows m = K[1,1][0] @ W, so

    out[b,h,w,o] = (sum_i x[b,h,w,i]) * m[o]

i.e. a channel-sum reduction followed by a rank-1 outer-product broadcast.
This is verified on the host at runtime; if the structure doesn't hold, an
exact (slow) numpy conv fallback is used instead.

Device kernel (per NeuronCore, data-parallel over the batch: 1 image/core):
the whole computation is ONE rank-1 matmul on TensorE.  The host shards each
image as x_T[c, pos] (a pure layout transpose, fused with the f32->bf16 cast;
rel tolerance is 2e-2, bf16 rounding costs ~2e-3 here and halves HBM traffic).
Per 2048-position tile:

    psum[o, pos] = sum_c A[c, o] * x_T[c, pos],   A[c, o] = m[o]  (replicated)

via 4 matmuls of N=512 (one PSUM bank each), then one PSUM->SBUF bf16 copy
(alternating VectorE/ScalarE so neither engine gates the DMA cadence), then a
contiguous store of out_T[c_out, pos].  The host transposes the result back
during the unshard.  Roofline: 2 x 16.75 MB per core over ~358 GB/s HBM.
"""

import sys

import numpy as np

for _p in ("/opt/trn_rl_repo", "/root/.axon_site/_ro/trn_rl_repo"):
    if _p not in sys.path:
        sys.path.insert(0, _p)

C = 128            # channels (cin == cout), also SBUF partitions
NPOS = 256 * 256   # positions per core (one image per core)
PB = 2048          # positions per tile
T = NPOS // PB     # 32 tiles
MM_N = 512         # matmul free dim = one fp32 PSUM bank
N_CORES = 8

_NC_CACHE = {}


def _build_nc():
    import concourse.bass as bass
    import concourse.bacc as bacc
    import concourse.tile as tile
    from concourse import mybir

    BF = mybir.dt.bfloat16
    nc = bacc.Bacc(None)
    x = nc.dram_tensor("x", [C, NPOS], BF, kind="ExternalInput")
    w = nc.dram_tensor("wsum", [C, C], BF, kind="ExternalInput")
    out = nc.dram_tensor("out", [C, NPOS], BF, kind="ExternalOutput")

    with tile.TileContext(nc) as tc:
        with (
            tc.tile_pool(name="xin", bufs=5) as xin_pool,
            tc.tile_pool(name="oout", bufs=5) as out_pool,
            tc.tile_pool(name="ps", bufs=2, space="PSUM") as psum_pool,
            tc.tile_pool(name="const", bufs=1) as const_pool,
        ):
            # Stationary A[c, o] = m[o] (32 KB), via the SWDGE ring so the SP
            # ring starts streaming x tiles immediately.
            wt = const_pool.tile([C, C], BF)
            nc.gpsimd.dma_start(out=wt[:], in_=w[:])

            # Loads ride the SP HWDGE ring, stores the ACT HWDGE ring: with
            # both directions split across the two HWDGE rings the combined
            # stream sustains ~420 GB/s (SBUF-fabric-bound, not the 358 GB/s
            # single-direction HBM figure).  1 MB loads keep the read stream
            # dense from t=0; a small-tile ramp at the END shortens drain.
            # PSUM holds 8 banks: chunks of 2048 fp32 x bufs=2.
            sizes = [2 * PB] * 15 + [PB, PB // 2, MM_N, MM_N]
            assert sum(sizes) == NPOS
            chunk_i = 0
            pos = 0
            for t, pb in enumerate(sizes):
                xt = xin_pool.tile([C, pb], BF)
                nc.sync.dma_start(out=xt[:], in_=x[:, pos:pos + pb])

                ot = out_pool.tile([C, pb], BF)
                for c0 in range(0, pb, PB):
                    cw = min(PB, pb - c0)
                    ps = psum_pool.tile([C, cw], mybir.dt.float32)
                    for k in range(cw // MM_N):
                        nc.tensor.matmul(
                            ps[:, k * MM_N:(k + 1) * MM_N],
                            wt[:],
                            xt[:, c0 + k * MM_N:c0 + (k + 1) * MM_N],
                            start=True,
                            stop=True,
                        )
                    if chunk_i % 2 == 0:
                        nc.vector.tensor_copy(out=ot[:, c0:c0 + cw], in_=ps[:])
                    else:
                        nc.scalar.copy(out=ot[:, c0:c0 + cw], in_=ps[:])
                    chunk_i += 1
                nc.scalar.dma_start(out=out[:, pos:pos + pb], in_=ot[:])
                pos += pb

    nc.finalize()
    return nc


def _get_nc():
    if "nc" not in _NC_CACHE:
        _NC_CACHE["nc"] = _build_nc()
    return _NC_CACHE["nc"]


def _fallback_numpy(X, K, b, Wd):
    """Exact general path: full 3x3 SAME conv + bias, then Dense. Only used if
    the staged inputs ever stop matching the one-hot-tap structure."""
    B, H, Wi, Ci = X.shape
    Co = Wd.shape[1]
    M = np.einsum("xyic,co->xyio", K, Wd).astype(np.float32)
    Xp = np.zeros((B, H + 2, Wi + 2, Ci), np.float32)
    Xp[:, 1:-1, 1:-1, :] = X
    out = np.zeros((B, H, Wi, Co), np.float32)
    for dx in range(3):
        for dy in range(3):
            out += Xp[:, dx : dx + H, dy : dy + Wi, :] @ M[dx, dy]
    out += b @ Wd
    return out.astype(np.float32)


def _install_ntff_hook():
    """Provide antenv.axon_hooks if the image lacks it (slim ctypes NTFF hook,
    same mechanism as trn_agent_boot.trn_boot._ntff_profile_via_ctypes)."""
    try:
        from antenv.axon_hooks import get_axon_ntff_profile_hook  # noqa: F401

        return
    except ImportError:
        pass

    import contextlib
    import ctypes
    import types

    so_path = "/opt/axon/libaxon_pjrt.so"
    lib = ctypes.CDLL(so_path)
    if not hasattr(lib, "axon_start_nrt_profile"):
        hook = None
    else:
        lib.axon_start_nrt_profile.argtypes = [
            ctypes.POINTER(ctypes.c_int64),
            ctypes.c_size_t,
        ]
        lib.axon_start_nrt_profile.restype = ctypes.c_int64
        lib.axon_stop_nrt_profile.argtypes = [ctypes.c_char_p]
        lib.axon_stop_nrt_profile.restype = ctypes.c_int64

        @contextlib.contextmanager
        def hook(output_dir, device_ids):
            import jax

            jax.devices()
            if device_ids:
                ids = (ctypes.c_int64 * len(device_ids))(*device_ids)
                rc = lib.axon_start_nrt_profile(ids, len(device_ids))
            else:
                rc = lib.axon_start_nrt_profile(None, 0)
            if rc != 0:
                raise RuntimeError(f"axon_start_nrt_profile rc={rc}")
            try:
                yield
            finally:
                n = lib.axon_stop_nrt_profile(str(output_dir).encode())
                print(f"ntff profile: {n} file(s) written to {output_dir}")

    mod = types.ModuleType("antenv.axon_hooks")
    mod.get_axon_ntff_profile_hook = lambda: hook
    mod.set_axon_ntff_profile_hook = lambda h: None
    sys.modules["antenv.axon_hooks"] = mod
    import antenv

    antenv.axon_hooks = mod


def _run_device(in_maps, trace=False, **kwargs):
    import concourse.bass_utils as bu

    if trace:
        _install_ntff_hook()
        # Zero-egress container: keep artifacts local instead of uploading.
        bu.upload_artifacts = lambda tmpdir: str(tmpdir)

    nc = _get_nc()
    return bu.run_bass_kernel_spmd(
        nc, in_maps, list(range(N_CORES)), trace=trace, **kwargs
    )


def _prepare(inputs, kernel, bias, W):
    X = np.ascontiguousarray(np.asarray(inputs, dtype=np.float32))
    K = np.asarray(kernel, dtype=np.float32)
    b = np.asarray(bias, dtype=np.float32)
    Wd = np.asarray(W, dtype=np.float32)

    structure_ok = (
        X.shape == (N_CORES, 256, 256, C)
        and K.shape == (3, 3, C, C)
        and Wd.shape == (C, C)
        and all(
            not np.any(K[dx, dy])
            for dx in range(3)
            for dy in range(3)
            if (dx, dy) != (1, 1)
        )
        and bool(np.all(K[1, 1] == K[1, 1][0:1, :]))
    )
    if not structure_ok:
        return None

    import ml_dtypes

    bf16 = ml_dtypes.bfloat16
    m = (K[1, 1][0:1, :] @ Wd)[0]          # (C,) folded rank-1 weight
    b_eff = (b @ Wd).astype(np.float32)    # (C,) folded bias (zeros in practice)
    wsum_rep = np.ascontiguousarray(np.broadcast_to(m.astype(bf16), (C, C)))
    # Shard layout: x_T[c, pos] per core (cast + transpose in one pass).
    Xf = X.reshape(N_CORES, NPOS, C)
    in_maps = [{"x": Xf[i].T.astype(bf16), "wsum": wsum_rep} for i in range(N_CORES)]
    return in_maps, b_eff


def _gather(res, b_eff):
    # Unshard: out_T[c, pos] bf16 -> out[pos, c] f32 per core, then stack.
    out = np.stack(
        [res.results[i]["out"].T.astype(np.float32) for i in range(N_CORES)]
    )
    out = out.reshape(N_CORES, 256, 256, C)
    if np.any(b_eff):
        out = (out + b_eff).astype(np.float32)
    return out


def kernel(inputs, kernel, bias, W):
    prep = _prepare(inputs, kernel, bias, W)
    if prep is None:
        return _fallback_numpy(
            np.asarray(inputs, np.float32),
            np.asarray(kernel, np.float32),
            np.asarray(bias, np.float32),
            np.asarray(W, np.float32),
        )
    in_maps, b_eff = prep

    try:
        res = _run_device(in_maps, trace=False)
    except Exception:
        return _fallback_numpy(
            np.asarray(inputs, np.float32),
            np.asarray(kernel, np.float32),
            np.asarray(bias, np.float32),
            np.asarray(W, np.float32),
        )
    return _gather(res, b_eff)


def kernel_traced(inputs, kernel, bias, W, **kwargs):
    """Like kernel(), but profiles on HW; returns (output, BassKernelResults)."""
    prep = _prepare(inputs, kernel, bias, W)
    assert prep is not None, "inputs do not match the staged structure"
    in_maps, b_eff = prep
    res = _run_device(in_maps, trace=True, **kwargs)
    return _gather(res, b_eff), res


# revision 19
# speedup vs baseline: 1.1573x; 1.1573x over previous
"""Trainium2 Bass kernel for ConvOffset: Conv2D(3x3, fixed one-hot-tap kernel) + Dense.

The staged conv kernel is zero everywhere except the center tap [1,1], which is
all-ones over (cin, cout).  Folding the conv kernel into the Dense weight W:

    out[b,h,w,o] = sum_i x[b,h,w,i] * M11[i,o] + bias @ W,
    M11[i,o]     = sum_c K[1,1,i,c] * W[c,o]

and because K[1,1] has identical rows (all-ones), M11 is rank-1 with identical
rows m = K[1,1][0] @ W, so

    out[b,h,w,o] = (sum_i x[b,h,w,i]) * m[o]

i.e. a channel-sum reduction followed by a rank-1 outer-product broadcast.
This is verified on the host at runtime; if the structure doesn't hold, an
exact (slow) numpy conv fallback is used instead.

Device kernel (per NeuronCore, data-parallel over the batch: 1 image/core):
the whole computation is ONE rank-1 matmul on TensorE.  The host shards each
image as x_T[c, pos] (a pure layout transpose, fused with the f32->bf16 cast;
rel tolerance is 2e-2, bf16 rounding costs ~2e-3 here and halves HBM traffic).
Per 2048-position tile:

    psum[o, pos] = sum_c A[c, o] * x_T[c, pos],   A[c, o] = m[o]  (replicated)

via 4 matmuls of N=512 (one PSUM bank each), then one PSUM->SBUF bf16 copy
(alternating VectorE/ScalarE so neither engine gates the DMA cadence), then a
contiguous store of out_T[c_out, pos].  The host transposes the result back
during the unshard.  Roofline: 2 x 16.75 MB per core over ~358 GB/s HBM.
"""

import sys

import numpy as np

for _p in ("/opt/trn_rl_repo", "/root/.axon_site/_ro/trn_rl_repo"):
    if _p not in sys.path:
        sys.path.insert(0, _p)

C = 128            # channels (cin == cout), also SBUF partitions
NPOS = 256 * 256   # positions per core (one image per core)
PB = 2048          # positions per tile
T = NPOS // PB     # 32 tiles
MM_N = 512         # matmul free dim = one fp32 PSUM bank
N_CORES = 8

_NC_CACHE = {}


def _build_nc():
    import concourse.bass as bass
    import concourse.bacc as bacc
    import concourse.tile as tile
    from concourse import mybir

    BF = mybir.dt.bfloat16
    nc = bacc.Bacc(None)
    x = nc.dram_tensor("x", [C, NPOS], BF, kind="ExternalInput")
    w = nc.dram_tensor("wsum", [C, C], BF, kind="ExternalInput")
    out = nc.dram_tensor("out", [C, NPOS], BF, kind="ExternalOutput")

    with tile.TileContext(nc) as tc:
        with (
            tc.tile_pool(name="xin", bufs=5) as xin_pool,
            tc.tile_pool(name="oout", bufs=5) as out_pool,
            tc.tile_pool(name="ps", bufs=2, space="PSUM") as psum_pool,
            tc.tile_pool(name="const", bufs=1) as const_pool,
        ):
            # Stationary A[c, o] = m[o] (32 KB), via the SWDGE ring so the SP
            # ring starts streaming x tiles immediately.
            wt = const_pool.tile([C, C], BF)
            nc.gpsimd.dma_start(out=wt[:], in_=w[:])

            # Loads ride the SP HWDGE ring, stores the ACT HWDGE ring: with
            # both directions split across the two HWDGE rings the combined
            # stream sustains ~420 GB/s (SBUF-fabric-bound, not the 358 GB/s
            # single-direction HBM figure).  0.5 MB tiles interleave best;
            # full-size loads from t=0 (stores have their own ring), small
            # tiles only at the END to shorten the store drain.
            sizes = [PB] * 31 + [MM_N] * 4
            assert sum(sizes) == NPOS
            pos = 0
            for t, pb in enumerate(sizes):
                xt = xin_pool.tile([C, pb], BF)
                nc.sync.dma_start(out=xt[:], in_=x[:, pos:pos + pb])

                ps = psum_pool.tile([C, pb], mybir.dt.float32)
                for k in range(pb // MM_N):
                    nc.tensor.matmul(
                        ps[:, k * MM_N:(k + 1) * MM_N],
                        wt[:],
                        xt[:, k * MM_N:(k + 1) * MM_N],
                        start=True,
                        stop=True,
                    )

                ot = out_pool.tile([C, pb], BF)
                if t % 2 == 0:
                    nc.vector.tensor_copy(out=ot[:], in_=ps[:])
                else:
                    nc.scalar.copy(out=ot[:], in_=ps[:])
                nc.scalar.dma_start(out=out[:, pos:pos + pb], in_=ot[:])
                pos += pb

    nc.finalize()
    return nc


def _get_nc():
    if "nc" not in _NC_CACHE:
        _NC_CACHE["nc"] = _build_nc()
    return _NC_CACHE["nc"]


def _fallback_numpy(X, K, b, Wd):
    """Exact general path: full 3x3 SAME conv + bias, then Dense. Only used if
    the staged inputs ever stop matching the one-hot-tap structure."""
    B, H, Wi, Ci = X.shape
    Co = Wd.shape[1]
    M = np.einsum("xyic,co->xyio", K, Wd).astype(np.float32)
    Xp = np.zeros((B, H + 2, Wi + 2, Ci), np.float32)
    Xp[:, 1:-1, 1:-1, :] = X
    out = np.zeros((B, H, Wi, Co), np.float32)
    for dx in range(3):
        for dy in range(3):
            out += Xp[:, dx : dx + H, dy : dy + Wi, :] @ M[dx, dy]
    out += b @ Wd
    return out.astype(np.float32)


def _install_ntff_hook():
    """Provide antenv.axon_hooks if the image lacks it (slim ctypes NTFF hook,
    same mechanism as trn_agent_boot.trn_boot._ntff_profile_via_ctypes)."""
    try:
        from antenv.axon_hooks import get_axon_ntff_profile_hook  # noqa: F401

        return
    except ImportError:
        pass

    import contextlib
    import ctypes
    import types

    so_path = "/opt/axon/libaxon_pjrt.so"
    lib = ctypes.CDLL(so_path)
    if not hasattr(lib, "axon_start_nrt_profile"):
        hook = None
    else:
        lib.axon_start_nrt_profile.argtypes = [
            ctypes.POINTER(ctypes.c_int64),
            ctypes.c_size_t,
        ]
        lib.axon_start_nrt_profile.restype = ctypes.c_int64
        lib.axon_stop_nrt_profile.argtypes = [ctypes.c_char_p]
        lib.axon_stop_nrt_profile.restype = ctypes.c_int64

        @contextlib.contextmanager
        def hook(output_dir, device_ids):
            import jax

            jax.devices()
            if device_ids:
                ids = (ctypes.c_int64 * len(device_ids))(*device_ids)
                rc = lib.axon_start_nrt_profile(ids, len(device_ids))
            else:
                rc = lib.axon_start_nrt_profile(None, 0)
            if rc != 0:
                raise RuntimeError(f"axon_start_nrt_profile rc={rc}")
            try:
                yield
            finally:
                n = lib.axon_stop_nrt_profile(str(output_dir).encode())
                print(f"ntff profile: {n} file(s) written to {output_dir}")

    mod = types.ModuleType("antenv.axon_hooks")
    mod.get_axon_ntff_profile_hook = lambda: hook
    mod.set_axon_ntff_profile_hook = lambda h: None
    sys.modules["antenv.axon_hooks"] = mod
    import antenv

    antenv.axon_hooks = mod


def _run_device(in_maps, trace=False, **kwargs):
    import concourse.bass_utils as bu

    if trace:
        _install_ntff_hook()
        # Zero-egress container: keep artifacts local instead of uploading.
        bu.upload_artifacts = lambda tmpdir: str(tmpdir)

    nc = _get_nc()
    return bu.run_bass_kernel_spmd(
        nc, in_maps, list(range(N_CORES)), trace=trace, **kwargs
    )


def _prepare(inputs, kernel, bias, W):
    X = np.ascontiguousarray(np.asarray(inputs, dtype=np.float32))
    K = np.asarray(kernel, dtype=np.float32)
    b = np.asarray(bias, dtype=np.float32)
    Wd = np.asarray(W, dtype=np.float32)

    structure_ok = (
        X.shape == (N_CORES, 256, 256, C)
        and K.shape == (3, 3, C, C)
        and Wd.shape == (C, C)
        and all(
            not np.any(K[dx, dy])
            for dx in range(3)
            for dy in range(3)
            if (dx, dy) != (1, 1)
        )
        and bool(np.all(K[1, 1] == K[1, 1][0:1, :]))
    )
    if not structure_ok:
        return None

    import ml_dtypes

    bf16 = ml_dtypes.bfloat16
    m = (K[1, 1][0:1, :] @ Wd)[0]          # (C,) folded rank-1 weight
    b_eff = (b @ Wd).astype(np.float32)    # (C,) folded bias (zeros in practice)
    wsum_rep = np.ascontiguousarray(np.broadcast_to(m.astype(bf16), (C, C)))
    # Shard layout: x_T[c, pos] per core (cast + transpose in one pass).
    Xf = X.reshape(N_CORES, NPOS, C)
    in_maps = [{"x": Xf[i].T.astype(bf16), "wsum": wsum_rep} for i in range(N_CORES)]
    return in_maps, b_eff


def _gather(res, b_eff):
    # Unshard: out_T[c, pos] bf16 -> out[pos, c] f32 per core, then stack.
    out = np.stack(
        [res.results[i]["out"].T.astype(np.float32) for i in range(N_CORES)]
    )
    out = out.reshape(N_CORES, 256, 256, C)
    if np.any(b_eff):
        out = (out + b_eff).astype(np.float32)
    return out


def kernel(inputs, kernel, bias, W):
    prep = _prepare(inputs, kernel, bias, W)
    if prep is None:
        return _fallback_numpy(
            np.asarray(inputs, np.float32),
            np.asarray(kernel, np.float32),
            np.asarray(bias, np.float32),
            np.asarray(W, np.float32),
        )
    in_maps, b_eff = prep

    try:
        res = _run_device(in_maps, trace=False)
    except Exception:
        return _fallback_numpy(
            np.asarray(inputs, np.float32),
            np.asarray(kernel, np.float32),
            np.asarray(bias, np.float32),
            np.asarray(W, np.float32),
        )
    return _gather(res, b_eff)


def kernel_traced(inputs, kernel, bias, W, **kwargs):
    """Like kernel(), but profiles on HW; returns (output, BassKernelResults)."""
    prep = _prepare(inputs, kernel, bias, W)
    assert prep is not None, "inputs do not match the staged structure"
    in_maps, b_eff = prep
    res = _run_device(in_maps, trace=True, **kwargs)
    return _gather(res, b_eff), res
